# revision 32
# baseline (speedup 1.0000x reference)
"""Contrastive-loss kernel for Trainium2 (8 NeuronCores, Bass/Tile).

Problem: x [32768,128] L2-normed rows, track_idxs [32768] in [0,512),
y [512,8,128] L2-normed. Reference computes S = exp(x @ y_sel.T / 0.3)
with y_sel = y.reshape(4096,128), pos[i,j] = (track_idxs[i] == j % 512),
num = sum(S[pos]), den = sum(S[~pos]), loss = -log(num/(den+1e-9)+1e-10).

Strategy (data-parallel over rows, per the sharding hint):
  - Host: stable-sort rows by track id, shard 4096 rows per core,
    pre-transpose to bf16 [128, 4096] (D on partitions). y replicated
    as bf16 y_sel^T [128, 4096].
  - Device per core: for each 128-row tile, matmul x^T.T @ y^T in bf16
    (PSUM fp32), then ScalarE activation Exp (in place on PSUM) with
    scale=1/temp and the fused accum_out giving per-partition sums
    (the "total").
  - Positive-pair sums ("num"): rows in a sorted 128-row tile span only
    a few track ids (window of W tracks starting at t0). A small second
    matmul computes dots against the W*8 candidate positive vectors,
    and an accumulating K=W matmul adds +50*temp to exactly the
    (row, candidate) pairs whose track matches (rank-W one-hot mask,
    prepared on host). Exp with bias=-50 then kills non-matches
    (exp(-50)~0) and leaves matches exact; accum_out gives num sums.
    Num psums for 8 row-tiles are batched into one 512-wide bank so a
    single activation covers them.
  - Host: sum per-core partials in float64, den = total - num,
    loss = -log(num/(den+1e-9)+1e-10).
"""

import numpy as np
import ml_dtypes

import concourse.bass as bass
import concourse.mybir as mybir
import concourse.tile as tile
from concourse import bacc
from concourse.bass_utils import run_bass_kernel_spmd

# Problem constants (hardcoded per harness contract).
N = 32768
D = 128
T = 512
Q = 8
NCORES = 8
R = N // NCORES            # rows per core = 4096
P = 128                    # partitions
NT = R // P                # row tiles per core = 32
TEMP = 0.3
EPS = 1e-9
EPS2 = 1e-10
SCALE = float(np.float32(1.0) / np.float32(TEMP))
MASK_BUMP = 50.0           # exponent bump for matched pairs
BIAS = -MASK_BUMP
CHUNK = 1024               # main psum chunk (2 banks)
MM_N = 512                 # matmul moving free dim (1 bank)
XSLAB = 1024               # xT DMA slab (rows)
DVE_NUM, DVE_DEN = 23, 64  # fraction of main chunks exp'd on DVE (0 = off)
# chosen so ACT time == DVE time under the pessimistic (drain-real) DVE
# cost model; robust either way.


def _to_dve(gi):
    return DVE_NUM and ((gi + 1) * DVE_NUM) % DVE_DEN < DVE_NUM

# Quadratic p(s) = (QA*s + QB)*s + QC with p(s)^8 ~ exp(s/TEMP) on
# s in [-1.06, 1.06]; coefficients minimax-rel fitted + calibrated so the
# summed (den) bias vanishes under the unit-vector dot distribution.
QA = 0.0875641213
QB = 0.42287001
QC = 0.999857945

_CACHE = {}
_QEXP = None


def _register_qexp():
    """Register the single-pass DVE op out[k] = p(in0[k])^8 with fused
    accum_out = sum(out). Runtime equivalent of appending to dve_ops.OPS."""
    global _QEXP
    if _QEXP is not None:
        return _QEXP
    from concourse import dve_ops
    from concourse.dve_spec import Spec, Src0, C0, C1, C2, sq, lower, AluOp
    from concourse.dve_uop import DveOpSpec

    name = "QEXP8_ANT"
    for op in dve_ops.OPS:
        if op.name == name:
            _QEXP = op
            return op

    body = sq(sq(sq((Src0 * C0 + C1) * Src0 + C2)))
    spec = Spec(
        body=body,
        accum=AluOp.ADD,
        reference=lambda in0, s0, s1, imm2: (
            ((in0 * s0 + s1) * in0 + imm2) ** 8
        ).astype(np.float32),
    )
    row = dve_ops._CUSTOM_DVE_ROW_BASE + len(dve_ops.OPS)
    shas = {}
    for ver in ("v3", "v4"):
        d = DveOpSpec(name=name, opcode=row, uops=lower(spec, ver=ver),
                      rd1_en=False)
        shas[ver] = d.sha(ver)
    op = dve_ops.DveOp(name, spec, subdim=False, uops_sha=shas)
    dve_ops.OPS.append(op)
    dve_ops._SUB_OPCODE_FOR_NAME[name] = row
    dve_ops.CUSTOM_DVE_SPECS[name] = spec
    _QEXP = op
    return op


def _build_program(W, to_dve=_to_dve):
    """Build the per-core Bass program. W = max tracks spanned by any
    128-row tile (global, so one program serves all cores / SPMD)."""
    W8 = W * Q
    assert W8 <= MM_N, f"track window {W} too wide for one-bank num psum"
    qexp = _register_qexp()
    nc = bacc.Bacc("TRN2", target_bir_lowering=False, debug=False,
                   num_devices=NCORES)

    xT_d = nc.dram_tensor("xT", (P, R), mybir.dt.bfloat16,
                          kind="ExternalInput").ap()
    yT_d = nc.dram_tensor("yT", (P, T * Q), mybir.dt.bfloat16,
                          kind="ExternalInput").ap()
    ywinT_d = nc.dram_tensor("ywinT", (P, NT * W8), mybir.dt.bfloat16,
                             kind="ExternalInput").ap()
    a50_d = nc.dram_tensor("a50", (W, R), mybir.dt.bfloat16,
                           kind="ExternalInput").ap()
    bm_d = nc.dram_tensor("bm", (W, W8), mybir.dt.bfloat16,
                          kind="ExternalInput").ap()
    n_chunks = T * Q // CHUNK  # 4
    # num batches: as many row-tiles per batched num-exp as fit in one bank
    numb = min(MM_N // W8, NT)
    groups = []
    start = 0
    while start < NT:
        groups.append((start, min(start + numb, NT)))
        start += numb
    tot_d = nc.dram_tensor("tot", (P, NT * n_chunks),
                           mybir.dt.float32, kind="ExternalOutput").ap()
    num_d = nc.dram_tensor("num", (P, len(groups)), mybir.dt.float32,
                           kind="ExternalOutput").ap()

    with tile.TileContext(nc) as tc:
        with (
            tc.tile_pool(name="const", bufs=1) as cp,
            tc.tile_pool(name="ps", bufs=4, space="PSUM") as ps,
        ):
            bias_s = cp.tile([P, 1], mybir.dt.float32)
            nc.any.memset(bias_s[:], BIAS)
            # Split loads (finely at the front) so first matmuls start early.
            x_slabs = [256, 256, 512] + [XSLAB] * ((R - XSLAB) // XSLAB)
            assert sum(x_slabs) == R
            x_off = [sum(x_slabs[:i]) for i in range(len(x_slabs))]
            xs_tiles = [cp.tile([P, x_slabs[i]], mybir.dt.bfloat16,
                                tag=f"xs{i}", name=f"xs{i}")
                        for i in range(len(x_slabs))]
            y00 = cp.tile([P, MM_N], mybir.dt.bfloat16)
            y01 = cp.tile([P, MM_N], mybir.dt.bfloat16)
            yh_tiles = [cp.tile([P, CHUNK], mybir.dt.bfloat16,
                                tag=f"yh{i}", name=f"yh{i}")
                        for i in range(1, n_chunks)]

            def rhs_of(c, m):
                if c == 0:
                    return (y00, y01)[m][:]
                return yh_tiles[c - 1][:, m * MM_N:(m + 1) * MM_N]

            # First wave on two parallel HWDGE queues (SP + ACT).
            nc.sync.dma_start(xs_tiles[0][:], xT_d[:, :x_slabs[0]])
            nc.scalar.dma_start(y00[:], yT_d[:, :MM_N])
            nc.scalar.dma_start(y01[:], yT_d[:, MM_N:2 * MM_N])
            nc.sync.dma_start(xs_tiles[1][:],
                              xT_d[:, x_off[1]:x_off[1] + x_slabs[1]])
            nc.scalar.dma_start(yh_tiles[0][:], yT_d[:, CHUNK:2 * CHUNK])
            for i in range(2, n_chunks):
                nc.sync.dma_start(yh_tiles[i - 1][:],
                                  yT_d[:, i * CHUNK:(i + 1) * CHUNK])
            for i in range(2, len(x_slabs)):
                nc.sync.dma_start(xs_tiles[i][:],
                                  xT_d[:, x_off[i]:x_off[i] + x_slabs[i]])
            ywinT_s = cp.tile([P, NT, W8], mybir.dt.bfloat16)
            a50_s = cp.tile([W, R], mybir.dt.bfloat16)
            bm_s = cp.tile([W, W8], mybir.dt.bfloat16)
            nc.gpsimd.dma_start(ywinT_s[:], ywinT_d.rearrange(
                "p (t w) -> p t w", w=W8))
            nc.gpsimd.dma_start(a50_s[:], a50_d)
            nc.gpsimd.dma_start(bm_s[:], bm_d)

            tot_s = cp.tile([P, NT * n_chunks], mybir.dt.float32)
            num_s = cp.tile([P, len(groups)], mybir.dt.float32)

            def lhsT_of(r):
                base = r * P
                for i, off in enumerate(x_off):
                    if off <= base < off + x_slabs[i]:
                        return xs_tiles[i][:, base - off:base - off + P]
                raise AssertionError(r)

            for g, (g0, g1) in enumerate(groups):
                for r in range(g0, g1):
                    lhsT = lhsT_of(r)
                    for c in range(n_chunks):
                        gi = r * n_chunks + c
                        psm = ps.tile([P, CHUNK], mybir.dt.float32, tag="ps")
                        for m in range(CHUNK // MM_N):
                            nc.tensor.matmul(
                                psm[:, m * MM_N:(m + 1) * MM_N],
                                lhsT,
                                rhs_of(c, m),
                                start=True, stop=True,
                            )
                        if to_dve(gi):
                            nc.vector._custom_dve(
                                qexp, out=psm[:], in0=psm[:],
                                s0=QA, s1=QB, imm2=QC,
                                accum_out=tot_s[:, gi:gi + 1],
                            )
                        else:
                            nc.scalar.activation(
                                psm[:], psm[:],
                                mybir.ActivationFunctionType.Exp,
                                scale=SCALE,
                                accum_out=tot_s[:, gi:gi + 1],
                            )
                # batched num for the row-tiles of this group
                psn = ps.tile([P, CHUNK], mybir.dt.float32, tag="ps")
                for r in range(g0, g1):
                    sl = slice((r - g0) * W8, (r - g0 + 1) * W8)
                    nc.tensor.matmul(
                        psn[:, sl], lhsT_of(r), ywinT_s[:, r],
                        start=True, stop=False,
                    )
                    nc.tensor.matmul(
                        psn[:, sl], a50_s[:, r * P:(r + 1) * P], bm_s[:],
                        start=False, stop=True,
                    )
                nc.scalar.activation(
                    psn[:, :(g1 - g0) * W8], psn[:, :(g1 - g0) * W8],
                    mybir.ActivationFunctionType.Exp,
                    scale=SCALE, bias=bias_s[:],
                    accum_out=num_s[:, g:g + 1],
                )

                if g1 == NT // 2:
                    half = NT * n_chunks // 2
                    nc.sync.dma_start(tot_d[:, :half], tot_s[:, :half])

            half = NT * n_chunks // 2
            nc.sync.dma_start(tot_d[:, half:], tot_s[:, half:])
            nc.sync.dma_start(num_d, num_s[:])

    nc.compile()
    return nc


def prepare_inputs(x, track_idxs, y):
    """Host-side layout prep: sort by track, shard, transpose, cast,
    and build the positive-window tensors."""
    order = np.argsort(track_idxs, kind="stable")
    xs = np.ascontiguousarray(x[order])
    ts = track_idxs[order].astype(np.int64)

    y_sel = np.ascontiguousarray(y.reshape(T * Q, D))  # row j = y[j//Q, j%Q]
    yT = np.ascontiguousarray(y_sel.T).astype(ml_dtypes.bfloat16)

    # Window span per 128-row tile (global max -> uniform SPMD program)
    t_first = ts[0::P]
    t_last = ts[P - 1::P]
    W = int((t_last - t_first).max()) + 1

    W8 = W * Q
    bm = np.zeros((W, W8), np.float32)
    for w in range(W):
        bm[w, w * Q:(w + 1) * Q] = 1.0
    bm = bm.astype(ml_dtypes.bfloat16)

    in_maps = []
    for c in range(NCORES):
        rows = slice(c * R, (c + 1) * R)
        xT = np.ascontiguousarray(xs[rows].T).astype(ml_dtypes.bfloat16)
        tsc = ts[rows]
        ywinT = np.zeros((P, NT, W8), np.float32)
        a50 = np.zeros((W, R), np.float32)
        for r in range(NT):
            t0 = int(tsc[r * P])
            for w in range(W):
                t = t0 + w
                if t >= T:
                    break
                # positives of track t are y_sel columns {t + T*k}
                ywinT[:, r, w * Q:(w + 1) * Q] = y_sel[t::T].T
            seg = tsc[r * P:(r + 1) * P] - t0
            a50[seg, np.arange(r * P, (r + 1) * P)] = MASK_BUMP * TEMP
        in_maps.append({
            "xT": xT,
            "yT": yT,
            "ywinT": np.ascontiguousarray(
                ywinT.reshape(P, NT * W8)).astype(ml_dtypes.bfloat16),
            "a50": a50.astype(ml_dtypes.bfloat16),
            "bm": bm,
        })
    return in_maps, W, track_idxs.dtype


def finalize(results):
    """Combine per-core partials into the scalar loss."""
    num = 0.0
    tot = 0.0
    for res in results:
        num += float(res["num"].astype(np.float64).sum())
        tot += float(res["tot"].astype(np.float64).sum())
    den = tot - num
    loss = -np.log(num / (den + EPS) + EPS2)
    return np.array([loss], dtype=np.float32)


def kernel(x, track_idxs, y):
    x = np.asarray(x)
    track_idxs = np.asarray(track_idxs)
    y = np.asarray(y)
    assert x.shape == (N, D) and y.shape == (T, Q, D)
    # Reference maps y through unique(track_idxs, size=T); with every
    # track present (true for this data) that is the identity.
    assert np.unique(track_idxs).size == T, "kernel assumes all tracks present"

    in_maps, W, _ = prepare_inputs(x, track_idxs, y)
    key = (W, DVE_NUM, DVE_DEN)
    if key not in _CACHE:
        _CACHE[key] = _build_program(W)
    nc = _CACHE[key]
    res = run_bass_kernel_spmd(nc, in_maps, core_ids=list(range(NCORES)))
    return finalize(res.results)
